# revision 4
# baseline (speedup 1.0000x reference)
"""Trainium2 Bass kernel for nn_AttentionFlow_Layer (BiDAF-style attention flow).

Sharding: data-parallel over batch B=8 — one batch element per NeuronCore, no
collectives. Per core the computation is

  sim[i,j] = ctx[i]·wc + q[j]·wq + (ctx[i]*wx)·q[j] (+ b_sim, which cancels in
             both softmaxes and is dropped) with -1e10 additive masks
  c2q      = softmax_j(sim) @ q
  q2c      = softmax_i(max_j sim) @ ctx
  x        = [ctx | c2q | ctx*c2q | ctx*q2c]            # [2048, 1024]
  out      = (relu(x @ W1 + b1) @ W2 + b2) * (1 - context_mask)

Activations are kept feature-major ("transposed", [d, tokens]) so both FFN
matmuls use the weight matrices directly as the stationary operand:
  hT = relu(W1.T @ xT + b1),  out[t, f] = hT[:, t]·W2[:, f] + b2
Row masking is applied once, at the final PSUM eviction, as a per-partition
(token) scalar multiply — masked rows come out exactly 0.

fp32 data throughout. Large matmuls run as float32r (single-pass full-rate
fp32): every f32r operand is produced by a DVE/ACT instruction whose output
access pattern is typed float32r, which applies the hardware rounding the BIR
verifier demands. Attention-statistics matmuls stay full fp32.
"""

import numpy as np

B, LC, LQ = 8, 2048, 128
D, D8 = 256, 1024
P = 128
NCH = LC // P  # 16 context chunks
NEG = np.float32(-1e10)
CLAMP = -30.0

_CACHE = {}


def _build_nc():
    import concourse.mybir as mybir
    import concourse.tile as tile
    from concourse import bacc
    from concourse.masks import make_identity

    f32 = mybir.dt.float32
    f32r = mybir.dt.float32r
    X = mybir.AxisListType.X
    Exp = mybir.ActivationFunctionType.Exp
    Relu = mybir.ActivationFunctionType.Relu
    Alu = mybir.AluOpType

    nc = bacc.Bacc("TRN2", target_bir_lowering=False, debug=False)

    # ---- DRAM I/O (per-core shard) ----
    ctxT_d = nc.dram_tensor("ctxT", [D, LC], f32, kind="ExternalInput")
    ctx_d = nc.dram_tensor("ctx", [LC, D], f32, kind="ExternalInput")
    q_d = nc.dram_tensor("q", [LQ, D], f32, kind="ExternalInput")
    qT_d = nc.dram_tensor("qT", [D, LQ], f32, kind="ExternalInput")
    cma2_d = nc.dram_tensor("cma2", [P, NCH], f32, kind="ExternalInput")
    m01c_d = nc.dram_tensor("m01c", [P, NCH], f32, kind="ExternalInput")
    qma_d = nc.dram_tensor("qma", [1, LQ], f32, kind="ExternalInput")
    wc_d = nc.dram_tensor("wc", [P, 2], f32, kind="ExternalInput")
    wq_d = nc.dram_tensor("wq", [P, 2], f32, kind="ExternalInput")
    wx_d = nc.dram_tensor("wx", [P, 2], f32, kind="ExternalInput")
    w1_d = nc.dram_tensor("w1", [D8, D8], f32, kind="ExternalInput")
    b1c_d = nc.dram_tensor("b1c", [P, 8], f32, kind="ExternalInput")
    w2_d = nc.dram_tensor("w2", [D8, D8], f32, kind="ExternalInput")
    b2_d = nc.dram_tensor("b2", [1, D8], f32, kind="ExternalInput")
    out_d = nc.dram_tensor("out", [LC, D8], f32, kind="ExternalOutput")

    def rr(ap):
        # float32r view of an f32 tile. Used on producer *outputs* (applies
        # hw rounding) and matmul operand inputs.
        return ap.bitcast(f32r)

    with tile.TileContext(nc) as tc:
        with tc.tile_pool(name="persist", bufs=1) as pp:
            ones = pp.tile([P, P], f32, tag="ones")
            nc.vector.memset(ones[:], 1.0)
            ident = pp.tile([P, P], f32, tag="ident")
            make_identity(nc, ident[:])

            # f32r-rounded FFN/attention operands (filled by copy/compute below)
            ctxT_r = pp.tile([P, 2, LC], f32, tag="ctxT_r")
            qx = pp.tile([P, 2, LQ], f32, tag="qx")
            q_r = pp.tile([P, 2, P], f32, tag="q_r")
            c2qT = pp.tile([P, 2, LC], f32, tag="c2qT")
            xb2 = pp.tile([P, 2, LC], f32, tag="xb2")
            xb3 = pp.tile([P, 2, LC], f32, tag="xb3")
            w1_sb = pp.tile([P, 8, D8], f32, tag="w1")
            w2_sb = pp.tile([P, 8, D8], f32, tag="w2")
            ones_r = pp.tile([1, P], f32, tag="ones_r")
            nc.vector.tensor_copy(rr(ones_r[:]), ones[0:1, :])
            b2_r = pp.tile([1, D8], f32, tag="b2_r")

            m01c_sb = pp.tile([P, NCH], f32, tag="m01c")
            nc.sync.dma_start(m01c_sb[:], m01c_d[:])
            b1c_sb = pp.tile([P, 8], f32, tag="b1c")
            nc.sync.dma_start(b1c_sb[:], b1c_d[:])

            # ---------------- loads + f32r rounding copies ----------------
            with (
                tc.tile_pool(name="wstage", bufs=2) as ws,
                tc.tile_pool(name="att_c", bufs=1) as ac,
            ):
                b2_st = ac.tile([1, D8], f32, tag="b2_st")
                nc.sync.dma_start(b2_st[:], b2_d[:])
                nc.vector.tensor_copy(rr(b2_r[:]), b2_st[:])

                for k in range(8):
                    w1_st = ws.tile([P, D8], f32, tag="w1_st")
                    nc.sync.dma_start(
                        w1_st[:], w1_d[k * P : (k + 1) * P, :]
                    )
                    nc.vector.tensor_copy(rr(w1_sb[:, k, :]), w1_st[:])
                for k in range(8):
                    w2_st = ws.tile([P, D8], f32, tag="w2_st")
                    nc.sync.dma_start(
                        w2_st[:], w2_d[k * P : (k + 1) * P, :]
                    )
                    nc.scalar.copy(rr(w2_sb[:, k, :]), w2_st[:])

                ctxT_sb = ac.tile([P, 2, LC], f32, tag="ctxT_sb")
                nc.sync.dma_start(
                    ctxT_sb[:], ctxT_d[:].rearrange("(o p) i -> p o i", p=P)
                )
                for kd in range(2):
                    nc.vector.tensor_copy(rr(ctxT_r[:, kd, :]), ctxT_sb[:, kd, :])

                qT_sb = ac.tile([P, 2, LQ], f32, tag="qT_sb")
                nc.sync.dma_start(
                    qT_sb[:], qT_d[:].rearrange("(o p) j -> p o j", p=P)
                )
                q_sb = ac.tile([P, 2, P], f32, tag="q_sb")
                nc.sync.dma_start(q_sb[:], q_d[:].rearrange("j (h d) -> j h d", d=P))
                for h in range(2):
                    nc.vector.tensor_copy(rr(q_r[:, h, :]), q_sb[:, h, :])
                qma_sb = ac.tile([1, LQ], f32, tag="qma_sb")
                nc.sync.dma_start(qma_sb[:], qma_d[:])
                cma2_sb = ac.tile([P, NCH], f32, tag="cma2_sb")
                nc.sync.dma_start(cma2_sb[:], cma2_d[:])
                wc_sb = ac.tile([P, 2], f32, tag="wc_sb")
                nc.sync.dma_start(wc_sb[:], wc_d[:])
                wq_sb = ac.tile([P, 2], f32, tag="wq_sb")
                nc.sync.dma_start(wq_sb[:], wq_d[:])
                wx_sb = ac.tile([P, 2], f32, tag="wx_sb")
                nc.sync.dma_start(wx_sb[:], wx_d[:])
                # qx = qT * wx, rounded
                for kd in range(2):
                    nc.vector.tensor_scalar_mul(
                        rr(qx[:, kd, :]), qT_sb[:, kd, :], wx_sb[:, kd : kd + 1]
                    )

                ctx_sb = ac.tile([P, NCH, D], f32, tag="ctx_sb")
                nc.sync.dma_start(
                    ctx_sb[:], ctx_d[:].rearrange("(o p) d -> p o d", p=P)
                )

                # ---------------- attention ----------------
                with (
                    tc.tile_pool(name="att_w", bufs=2) as aw,
                    tc.tile_pool(name="ps_s", bufs=2, space="PSUM") as pss,
                    tc.tile_pool(name="ps_t", bufs=2, space="PSUM") as pst,
                    tc.tile_pool(name="ps_cq", bufs=2, space="PSUM") as pscq,
                    tc.tile_pool(name="ps_sm", bufs=2, space="PSUM") as psm,
                ):
                    # sq' = q @ wq + qmask_add  -> [1, LQ]   (full fp32)
                    ps_sq = psm.tile([P, 512], f32, tag="psmall", name="ps_sq")
                    nc.tensor.matmul(
                        ps_sq[0:1, :LQ], wq_sb[:, 0:1], qT_sb[:, 0, :],
                        start=True, stop=False,
                    )
                    nc.tensor.matmul(
                        ps_sq[0:1, :LQ], wq_sb[:, 1:2], qT_sb[:, 1, :],
                        start=False, stop=False,
                    )
                    nc.tensor.matmul(
                        ps_sq[0:1, :LQ], ones[0:1, 0:1], qma_sb[0:1, :],
                        start=False, stop=True,
                    )
                    sqp = aw.tile([1, LQ], f32, tag="sqp", bufs=1)
                    nc.scalar.copy(rr(sqp[:]), ps_sq[0:1, :LQ])

                    scm = aw.tile([P, NCH], f32, tag="scm", bufs=1)
                    m_all = aw.tile([P, NCH], f32, tag="m_all", bufs=1)
                    s_all = aw.tile([P, NCH], f32, tag="s_all", bufs=1)
                    rec_all = aw.tile([P, NCH], f32, tag="rec_all", bufs=1)

                    atile = None
                    for c in range(NCH):
                        cs = slice(c * P, (c + 1) * P)
                        # sc + cmask_add, column layout [128, 1] (full fp32)
                        ps_sc = psm.tile([P, 512], f32, tag="psmall", name="ps_sc")
                        nc.tensor.matmul(
                            ps_sc[:, 0:1], ctxT_sb[:, 0, cs], wc_sb[:, 0:1],
                            start=True, stop=False,
                        )
                        nc.tensor.matmul(
                            ps_sc[:, 0:1], ctxT_sb[:, 1, cs], wc_sb[:, 1:2],
                            start=False, stop=True,
                        )
                        nc.vector.tensor_tensor(
                            scm[:, c : c + 1], ps_sc[:, 0:1], cma2_sb[:, c : c + 1],
                            Alu.add,
                        )

                        # sim chunk [128 i, 128 j] = ctxT'.T @ qx + sq'[j]
                        ps_s = pss.tile([P, P], f32, tag="ps_s")
                        nc.tensor.matmul(
                            ps_s[:], rr(ctxT_r[:, 0, cs]), rr(qx[:, 0, :]),
                            start=True, stop=False,
                        )
                        nc.tensor.matmul(
                            ps_s[:], rr(ctxT_r[:, 1, cs]), rr(qx[:, 1, :]),
                            start=False, stop=False,
                        )
                        nc.tensor.matmul(
                            ps_s[:], rr(ones_r[0:1, :]), rr(sqp[0:1, :]),
                            start=False, stop=True,
                        )
                        # + sc[i] (per-partition), clamp at -30
                        smc = aw.tile([P, P], f32, tag="smc")
                        nc.vector.tensor_scalar(
                            smc[:], ps_s[:], scm[:, c : c + 1], CLAMP, Alu.add, Alu.max
                        )
                        nc.vector.reduce_max(m_all[:, c : c + 1], smc[:], axis=X)
                        U = aw.tile([P, P], f32, tag="U")
                        nc.scalar.activation(
                            U[:], smc[:], Exp, accum_out=s_all[:, c : c + 1]
                        )
                        nc.vector.reciprocal(rec_all[:, c : c + 1], s_all[:, c : c + 1])
                        A = aw.tile([P, P], f32, tag="A")
                        nc.vector.tensor_scalar_mul(A[:], U[:], rec_all[:, c : c + 1])
                        ps_t = pst.tile([P, P], f32, tag="ps_t")
                        nc.tensor.transpose(ps_t[:], A[:], ident[:])
                        g, cc = divmod(c, 4)
                        if cc == 0:
                            atile = aw.tile([P, 512], f32, tag="atile")
                        nc.scalar.copy(rr(atile[:, cc * P : (cc + 1) * P]), ps_t[:])
                        if cc == 3:
                            gs = slice(g * 512, (g + 1) * 512)
                            for h in range(2):
                                ps_cq = pscq.tile([P, 512], f32, tag="ps_cq")
                                nc.tensor.matmul(
                                    ps_cq[:], rr(q_r[:, h, :]), rr(atile[:]),
                                    start=True, stop=True,
                                )
                                nc.vector.tensor_copy(rr(c2qT[:, h, gs]), ps_cq[:])

                    # ---- q2c: softmax_i(max_j sim) @ ctx (full fp32) ----
                    E = aw.tile([P, NCH], f32, tag="E", bufs=1)
                    Es = aw.tile([P, 1], f32, tag="Es", bufs=1)
                    nc.scalar.activation(E[:], m_all[:], Exp, accum_out=Es[:])
                    ps_S = psm.tile([P, 512], f32, tag="psmall", name="ps_S")
                    nc.tensor.matmul(
                        ps_S[0:1, 0:1], Es[:], ones[:, 0:1], start=True, stop=True
                    )
                    invS = aw.tile([1, 1], f32, tag="invS", bufs=1)
                    nc.vector.reciprocal(invS[:], ps_S[0:1, 0:1])

                    ps_q2c = psm.tile([P, 512], f32, tag="psmall", name="ps_q2c")
                    for c in range(NCH):
                        nc.tensor.matmul(
                            ps_q2c[0:1, :D], E[:, c : c + 1], ctx_sb[:, c, :],
                            start=(c == 0), stop=(c == NCH - 1),
                        )
                    q2cn = aw.tile([1, D], f32, tag="q2cn", bufs=1)
                    nc.vector.tensor_scalar_mul(q2cn[:], ps_q2c[0:1, :D], invS[:])

                    ps_qt = pst.tile([P, P], f32, tag="ps_t", name="ps_qt")
                    for h in range(2):
                        nc.tensor.matmul(
                            ps_qt[:, h : h + 1], q2cn[0:1, h * P : (h + 1) * P],
                            ones[0:1, 0:1], start=True, stop=True,
                        )
                    q2cT = aw.tile([P, 2], f32, tag="q2cT", bufs=1)
                    nc.scalar.copy(q2cT[:], ps_qt[:, 0:2])

                    # ---- x blocks 2/3 (rounded for the FFN) ----
                    for kd in range(2):
                        nc.vector.tensor_tensor(
                            rr(xb2[:, kd, :]), ctxT_r[:, kd, :], c2qT[:, kd, :],
                            Alu.mult,
                        )
                        nc.vector.tensor_scalar_mul(
                            rr(xb3[:, kd, :]), ctxT_r[:, kd, :], q2cT[:, kd : kd + 1]
                        )

            # ---------------- FFN ----------------
            xs = [
                ctxT_r[:, 0, :], ctxT_r[:, 1, :],
                c2qT[:, 0, :], c2qT[:, 1, :],
                xb2[:, 0, :], xb2[:, 1, :],
                xb3[:, 0, :], xb3[:, 1, :],
            ]
            with (
                tc.tile_pool(name="ffn_sb", bufs=2) as fp,
                tc.tile_pool(name="ffn_ot", bufs=3) as op_,
                tc.tile_pool(name="psum1", bufs=3, space="PSUM") as pp1,
                tc.tile_pool(name="psum2", bufs=3, space="PSUM") as pp2,
            ):
                for nt in range(4):
                    ts512 = slice(nt * 512, (nt + 1) * 512)
                    hT = fp.tile([P, 8, 512], f32, tag="hT")
                    for mf in range(8):
                        ms = slice(mf * P, (mf + 1) * P)
                        ps1 = pp1.tile([P, 512], f32, tag="ps1")
                        for k in range(8):
                            nc.tensor.matmul(
                                ps1[:], rr(w1_sb[:, k, ms]), rr(xs[k][:, ts512]),
                                start=(k == 0), stop=(k == 7),
                            )
                        nc.scalar.activation(
                            rr(hT[:, mf, :]), ps1[:], Relu,
                            bias=b1c_sb[:, mf : mf + 1],
                        )
                    for t2 in range(4):
                        tg = nt * 4 + t2
                        for nf in range(2):
                            fs = slice(nf * 512, (nf + 1) * 512)
                            ps2 = pp2.tile([P, 512], f32, tag="ps2")
                            for k in range(8):
                                nc.tensor.matmul(
                                    ps2[:], rr(hT[:, k, t2 * P : (t2 + 1) * P]),
                                    rr(w2_sb[:, k, fs]), start=(k == 0), stop=False,
                                )
                            nc.tensor.matmul(
                                ps2[:], rr(ones_r[0:1, :]), rr(b2_r[0:1, fs]),
                                start=False, stop=True,
                            )
                            ot = op_.tile([P, 512], f32, tag="ot")
                            nc.vector.tensor_scalar_mul(
                                ot[:], ps2[:], m01c_sb[:, tg : tg + 1]
                            )
                            nc.sync.dma_start(
                                out_d[tg * P : (tg + 1) * P, fs], ot[:]
                            )

    nc.compile()
    return nc


def _get_nc():
    if "nc" not in _CACHE:
        _CACHE["nc"] = _build_nc()
    return _CACHE["nc"]


def _prep_inputs(inputs):
    ctx_all = np.ascontiguousarray(np.asarray(inputs["context_info"], np.float32))
    q_all = np.ascontiguousarray(np.asarray(inputs["query_info"], np.float32))
    cmask = np.asarray(inputs["context_mask"])
    qmask = np.asarray(inputs["query_mask"])
    w_sim = np.asarray(inputs["w_sim"], np.float32)
    w1 = np.ascontiguousarray(np.asarray(inputs["w_inner"], np.float32))
    b1c = np.ascontiguousarray(
        np.asarray(inputs["b_inner"], np.float32).reshape(8, P).T
    )
    w2 = np.ascontiguousarray(np.asarray(inputs["w_out"], np.float32))
    b2 = np.asarray(inputs["b_out"], np.float32).reshape(1, D8).copy()
    # b_sim is dropped: an additive constant cancels in both softmaxes.
    wc = np.ascontiguousarray(w_sim[:D].reshape(2, P).T)
    wq = np.ascontiguousarray(w_sim[D : 2 * D].reshape(2, P).T)
    wx = np.ascontiguousarray(w_sim[2 * D :].reshape(2, P).T)
    in_maps = []
    for b in range(B):
        cma = np.where(cmask[b], NEG, np.float32(0)).astype(np.float32)
        m01 = np.where(cmask[b], np.float32(0), np.float32(1)).astype(np.float32)
        in_maps.append(
            {
                "ctxT": np.ascontiguousarray(ctx_all[b].T),
                "ctx": ctx_all[b],
                "q": q_all[b],
                "qT": np.ascontiguousarray(q_all[b].T),
                "cma2": np.ascontiguousarray(cma.reshape(NCH, P).T),
                "m01c": np.ascontiguousarray(m01.reshape(NCH, P).T),
                "qma": np.where(qmask[b], NEG, np.float32(0))
                .astype(np.float32)
                .reshape(1, LQ),
                "wc": wc,
                "wq": wq,
                "wx": wx,
                "w1": w1,
                "b1c": b1c,
                "w2": w2,
                "b2": b2,
            }
        )
    return in_maps


def run(inputs, **kwargs):
    from concourse.bass_utils import run_bass_kernel_spmd

    nc = _get_nc()
    in_maps = _prep_inputs(inputs)
    res = run_bass_kernel_spmd(nc, in_maps, core_ids=list(range(B)), **kwargs)
    out = np.stack([r["out"] for r in res.results], axis=0)
    return out, res


def kernel(**inputs):
    out, _ = run(inputs)
    return out


# revision 6
# speedup vs baseline: 1.1216x; 1.1216x over previous
"""Trainium2 Bass kernel for nn_AttentionFlow_Layer (BiDAF-style attention flow).

Sharding: data-parallel over batch B=8 — one batch element per NeuronCore, no
collectives. Per core the computation is

  sim[i,j] = ctx[i]·wc + q[j]·wq + (ctx[i]*wx)·q[j] (+ b_sim, which cancels in
             both softmaxes and is dropped) with -1e10 additive masks
  c2q      = softmax_j(sim) @ q
  q2c      = softmax_i(max_j sim) @ ctx
  x        = [ctx | c2q | ctx*c2q | ctx*q2c]            # [2048, 1024]
  out      = (relu(x @ W1 + b1) @ W2 + b2) * (1 - context_mask)

Activations are kept feature-major ("transposed", [d, tokens]) so both FFN
matmuls use the weight matrices directly as the stationary operand:
  hT = relu(W1.T @ xT + b1),  out[t, f] = hT[:, t]·W2[:, f] + b2
Row masking is applied once, at the final PSUM eviction, as a per-partition
(token) scalar multiply — masked rows come out exactly 0.

fp32 data throughout. Large matmuls run as float32r (single-pass full-rate
fp32): every f32r operand is produced by a DVE/ACT instruction whose output
access pattern is typed float32r, which applies the hardware rounding the BIR
verifier demands. Attention-statistics matmuls stay full fp32.
"""

import numpy as np

B, LC, LQ = 8, 2048, 128
D, D8 = 256, 1024
P = 128
NCH = LC // P  # 16 context chunks
NEG = np.float32(-1e10)
CLAMP = -30.0

_CACHE = {}


def _build_nc():
    import concourse.mybir as mybir
    import concourse.tile as tile
    from concourse import bacc
    from concourse.masks import make_identity

    f32 = mybir.dt.float32
    f32r = mybir.dt.float32r
    X = mybir.AxisListType.X
    Exp = mybir.ActivationFunctionType.Exp
    Relu = mybir.ActivationFunctionType.Relu
    Alu = mybir.AluOpType

    nc = bacc.Bacc("TRN2", target_bir_lowering=False, debug=False)

    # ---- DRAM I/O (per-core shard) ----
    ctxT_d = nc.dram_tensor("ctxT", [D, LC], f32, kind="ExternalInput")
    ctx_d = nc.dram_tensor("ctx", [LC, D], f32, kind="ExternalInput")
    q_d = nc.dram_tensor("q", [LQ, D], f32, kind="ExternalInput")
    qT_d = nc.dram_tensor("qT", [D, LQ], f32, kind="ExternalInput")
    cma2_d = nc.dram_tensor("cma2", [P, NCH], f32, kind="ExternalInput")
    m01c_d = nc.dram_tensor("m01c", [P, NCH], f32, kind="ExternalInput")
    qma_d = nc.dram_tensor("qma", [1, LQ], f32, kind="ExternalInput")
    wc_d = nc.dram_tensor("wc", [P, 2], f32, kind="ExternalInput")
    wq_d = nc.dram_tensor("wq", [P, 2], f32, kind="ExternalInput")
    wx_d = nc.dram_tensor("wx", [P, 2], f32, kind="ExternalInput")
    w1_d = nc.dram_tensor("w1", [D8, D8], f32, kind="ExternalInput")
    b1c_d = nc.dram_tensor("b1c", [P, 8], f32, kind="ExternalInput")
    w2_d = nc.dram_tensor("w2", [D8, D8], f32, kind="ExternalInput")
    b2_d = nc.dram_tensor("b2", [1, D8], f32, kind="ExternalInput")
    out_d = nc.dram_tensor("out", [LC, D8], f32, kind="ExternalOutput")

    def rr(ap):
        # float32r view of an f32 tile. Used on producer *outputs* (applies
        # hw rounding) and matmul operand inputs.
        return ap.bitcast(f32r)

    with tile.TileContext(nc) as tc:
        with tc.tile_pool(name="persist", bufs=1) as pp:
            ones = pp.tile([P, P], f32, tag="ones")
            nc.vector.memset(ones[:], 1.0)
            ident = pp.tile([P, P], f32, tag="ident")
            make_identity(nc, ident[:])

            # f32r-rounded FFN/attention operands (filled by copy/compute below)
            ctxT_r = pp.tile([P, 2, LC], f32, tag="ctxT_r")
            qx = pp.tile([P, 2, LQ], f32, tag="qx")
            q_r = pp.tile([P, 2, P], f32, tag="q_r")
            c2qT = pp.tile([P, 2, LC], f32, tag="c2qT")
            xb2 = pp.tile([P, 2, LC], f32, tag="xb2")
            xb3 = pp.tile([P, 2, LC], f32, tag="xb3")
            w1_sb = pp.tile([P, 8, D8], f32, tag="w1")
            w2_sb = pp.tile([P, 8, D8], f32, tag="w2")
            ones_r = pp.tile([1, P], f32, tag="ones_r")
            nc.vector.tensor_copy(rr(ones_r[:]), ones[0:1, :])
            b2_r = pp.tile([1, D8], f32, tag="b2_r")

            m01c_sb = pp.tile([P, NCH], f32, tag="m01c")
            nc.sync.dma_start(m01c_sb[:], m01c_d[:])
            b1c_sb = pp.tile([P, 8], f32, tag="b1c")
            nc.sync.dma_start(b1c_sb[:], b1c_d[:])

            # ---------------- loads + f32r rounding copies ----------------
            # Order matters: attention inputs first so the PE can start while
            # the 8 MB of FFN weights stream in behind them.
            with (
                tc.tile_pool(name="wstage", bufs=2) as ws,
                tc.tile_pool(name="att_c", bufs=1) as ac,
            ):
                # PE warmup in the DMA shadow: ~10us of throwaway matmuls so
                # the HAM clock gate reaches 8/8 before the real work lands.
                with tc.tile_pool(name="ps_w", bufs=1, space="PSUM") as psw:
                    ps_warm = psw.tile([P, P], f32, tag="ps_warm")
                    for _ in range(24):
                        nc.tensor.matmul(
                            ps_warm[:], ones[:], ident[:], start=True, stop=True
                        )

                qT_sb = ac.tile([P, 2, LQ], f32, tag="qT_sb")
                nc.sync.dma_start(
                    qT_sb[:], qT_d[:].rearrange("(o p) j -> p o j", p=P)
                )
                q_sb = ac.tile([P, 2, P], f32, tag="q_sb")
                nc.sync.dma_start(q_sb[:], q_d[:].rearrange("j (h d) -> j h d", d=P))
                qma_sb = ac.tile([1, LQ], f32, tag="qma_sb")
                nc.sync.dma_start(qma_sb[:], qma_d[:])
                cma2_sb = ac.tile([P, NCH], f32, tag="cma2_sb")
                nc.sync.dma_start(cma2_sb[:], cma2_d[:])
                wc_sb = ac.tile([P, 2], f32, tag="wc_sb")
                nc.sync.dma_start(wc_sb[:], wc_d[:])
                wq_sb = ac.tile([P, 2], f32, tag="wq_sb")
                nc.sync.dma_start(wq_sb[:], wq_d[:])
                wx_sb = ac.tile([P, 2], f32, tag="wx_sb")
                nc.sync.dma_start(wx_sb[:], wx_d[:])
                ctxT_sb = ac.tile([P, 2, LC], f32, tag="ctxT_sb")
                nc.sync.dma_start(
                    ctxT_sb[:], ctxT_d[:].rearrange("(o p) i -> p o i", p=P)
                )
                for kd in range(2):
                    nc.vector.tensor_copy(rr(ctxT_r[:, kd, :]), ctxT_sb[:, kd, :])
                for h in range(2):
                    nc.vector.tensor_copy(rr(q_r[:, h, :]), q_sb[:, h, :])
                # qx = qT * wx, rounded
                for kd in range(2):
                    nc.vector.tensor_scalar_mul(
                        rr(qx[:, kd, :]), qT_sb[:, kd, :], wx_sb[:, kd : kd + 1]
                    )

                ctx_sb = ac.tile([P, NCH, D], f32, tag="ctx_sb")
                nc.sync.dma_start(
                    ctx_sb[:], ctx_d[:].rearrange("(o p) d -> p o d", p=P)
                )

                b2_st = ac.tile([1, D8], f32, tag="b2_st")
                nc.sync.dma_start(b2_st[:], b2_d[:])
                nc.vector.tensor_copy(rr(b2_r[:]), b2_st[:])

                for k in range(8):
                    w1_st = ws.tile([P, D8], f32, tag="w1_st")
                    nc.sync.dma_start(
                        w1_st[:], w1_d[k * P : (k + 1) * P, :]
                    )
                    nc.vector.tensor_copy(rr(w1_sb[:, k, :]), w1_st[:])
                for k in range(8):
                    w2_st = ws.tile([P, D8], f32, tag="w2_st")
                    nc.sync.dma_start(
                        w2_st[:], w2_d[k * P : (k + 1) * P, :]
                    )
                    nc.scalar.copy(rr(w2_sb[:, k, :]), w2_st[:])

                # ---------------- attention ----------------
                with (
                    tc.tile_pool(name="att_w", bufs=2) as aw,
                    tc.tile_pool(name="ps_s", bufs=2, space="PSUM") as pss,
                    tc.tile_pool(name="ps_t", bufs=2, space="PSUM") as pst,
                    tc.tile_pool(name="ps_cq", bufs=2, space="PSUM") as pscq,
                    tc.tile_pool(name="ps_sm", bufs=2, space="PSUM") as psm,
                ):
                    # sq' = q @ wq + qmask_add  -> [1, LQ]   (full fp32)
                    ps_sq = psm.tile([P, 512], f32, tag="psmall", name="ps_sq")
                    nc.tensor.matmul(
                        ps_sq[0:1, :LQ], wq_sb[:, 0:1], qT_sb[:, 0, :],
                        start=True, stop=False,
                    )
                    nc.tensor.matmul(
                        ps_sq[0:1, :LQ], wq_sb[:, 1:2], qT_sb[:, 1, :],
                        start=False, stop=False,
                    )
                    nc.tensor.matmul(
                        ps_sq[0:1, :LQ], ones[0:1, 0:1], qma_sb[0:1, :],
                        start=False, stop=True,
                    )
                    sqp = aw.tile([1, LQ], f32, tag="sqp", bufs=1)
                    nc.scalar.copy(rr(sqp[:]), ps_sq[0:1, :LQ])

                    scm = aw.tile([P, NCH], f32, tag="scm", bufs=1)
                    m_all = aw.tile([P, NCH], f32, tag="m_all", bufs=1)
                    s_all = aw.tile([P, NCH], f32, tag="s_all", bufs=1)
                    rec_all = aw.tile([P, NCH], f32, tag="rec_all", bufs=1)

                    atile = None
                    for c in range(NCH):
                        cs = slice(c * P, (c + 1) * P)
                        # sc + cmask_add, column layout [128, 1] (full fp32)
                        ps_sc = psm.tile([P, 512], f32, tag="psmall", name="ps_sc")
                        nc.tensor.matmul(
                            ps_sc[:, 0:1], ctxT_sb[:, 0, cs], wc_sb[:, 0:1],
                            start=True, stop=False,
                        )
                        nc.tensor.matmul(
                            ps_sc[:, 0:1], ctxT_sb[:, 1, cs], wc_sb[:, 1:2],
                            start=False, stop=True,
                        )
                        nc.vector.tensor_tensor(
                            scm[:, c : c + 1], ps_sc[:, 0:1], cma2_sb[:, c : c + 1],
                            Alu.add,
                        )

                        # sim chunk [128 i, 128 j] = ctxT'.T @ qx + sq'[j]
                        ps_s = pss.tile([P, P], f32, tag="ps_s")
                        nc.tensor.matmul(
                            ps_s[:], rr(ctxT_r[:, 0, cs]), rr(qx[:, 0, :]),
                            start=True, stop=False,
                        )
                        nc.tensor.matmul(
                            ps_s[:], rr(ctxT_r[:, 1, cs]), rr(qx[:, 1, :]),
                            start=False, stop=False,
                        )
                        nc.tensor.matmul(
                            ps_s[:], rr(ones_r[0:1, :]), rr(sqp[0:1, :]),
                            start=False, stop=True,
                        )
                        # + sc[i] (per-partition), clamp at -30
                        smc = aw.tile([P, P], f32, tag="smc")
                        nc.vector.tensor_scalar(
                            smc[:], ps_s[:], scm[:, c : c + 1], CLAMP, Alu.add, Alu.max
                        )
                        nc.vector.reduce_max(m_all[:, c : c + 1], smc[:], axis=X)
                        U = aw.tile([P, P], f32, tag="U")
                        nc.scalar.activation(
                            U[:], smc[:], Exp, accum_out=s_all[:, c : c + 1]
                        )
                        nc.vector.reciprocal(rec_all[:, c : c + 1], s_all[:, c : c + 1])
                        A = aw.tile([P, P], f32, tag="A")
                        nc.vector.tensor_scalar_mul(A[:], U[:], rec_all[:, c : c + 1])
                        ps_t = pst.tile([P, P], f32, tag="ps_t")
                        nc.tensor.transpose(ps_t[:], A[:], ident[:])
                        g, cc = divmod(c, 4)
                        if cc == 0:
                            atile = aw.tile([P, 512], f32, tag="atile")
                        nc.scalar.copy(rr(atile[:, cc * P : (cc + 1) * P]), ps_t[:])
                        if cc == 3:
                            gs = slice(g * 512, (g + 1) * 512)
                            for h in range(2):
                                ps_cq = pscq.tile([P, 512], f32, tag="ps_cq")
                                nc.tensor.matmul(
                                    ps_cq[:], rr(q_r[:, h, :]), rr(atile[:]),
                                    start=True, stop=True,
                                )
                                nc.vector.tensor_copy(rr(c2qT[:, h, gs]), ps_cq[:])

                    # ---- q2c: softmax_i(max_j sim) @ ctx (full fp32) ----
                    E = aw.tile([P, NCH], f32, tag="E", bufs=1)
                    Es = aw.tile([P, 1], f32, tag="Es", bufs=1)
                    nc.scalar.activation(E[:], m_all[:], Exp, accum_out=Es[:])
                    ps_S = psm.tile([P, 512], f32, tag="psmall", name="ps_S")
                    nc.tensor.matmul(
                        ps_S[0:1, 0:1], Es[:], ones[:, 0:1], start=True, stop=True
                    )
                    invS = aw.tile([1, 1], f32, tag="invS", bufs=1)
                    nc.vector.reciprocal(invS[:], ps_S[0:1, 0:1])

                    ps_q2c = psm.tile([P, 512], f32, tag="psmall", name="ps_q2c")
                    for c in range(NCH):
                        nc.tensor.matmul(
                            ps_q2c[0:1, :D], E[:, c : c + 1], ctx_sb[:, c, :],
                            start=(c == 0), stop=(c == NCH - 1),
                        )
                    q2cn = aw.tile([1, D], f32, tag="q2cn", bufs=1)
                    nc.vector.tensor_scalar_mul(q2cn[:], ps_q2c[0:1, :D], invS[:])

                    ps_qt = pst.tile([P, P], f32, tag="ps_t", name="ps_qt")
                    for h in range(2):
                        nc.tensor.matmul(
                            ps_qt[:, h : h + 1], q2cn[0:1, h * P : (h + 1) * P],
                            ones[0:1, 0:1], start=True, stop=True,
                        )
                    q2cT = aw.tile([P, 2], f32, tag="q2cT", bufs=1)
                    nc.scalar.copy(q2cT[:], ps_qt[:, 0:2])

                    # ---- x blocks 2/3 (rounded for the FFN) ----
                    for kd in range(2):
                        nc.vector.tensor_tensor(
                            rr(xb2[:, kd, :]), ctxT_r[:, kd, :], c2qT[:, kd, :],
                            Alu.mult,
                        )
                        nc.vector.tensor_scalar_mul(
                            rr(xb3[:, kd, :]), ctxT_r[:, kd, :], q2cT[:, kd : kd + 1]
                        )

            # ---------------- FFN ----------------
            xs = [
                ctxT_r[:, 0, :], ctxT_r[:, 1, :],
                c2qT[:, 0, :], c2qT[:, 1, :],
                xb2[:, 0, :], xb2[:, 1, :],
                xb3[:, 0, :], xb3[:, 1, :],
            ]
            with (
                tc.tile_pool(name="ffn_sb", bufs=2) as fp,
                tc.tile_pool(name="ffn_ot", bufs=3) as op_,
                tc.tile_pool(name="psum1", bufs=3, space="PSUM") as pp1,
                tc.tile_pool(name="psum2", bufs=3, space="PSUM") as pp2,
            ):
                for nt in range(4):
                    ts512 = slice(nt * 512, (nt + 1) * 512)
                    hT = fp.tile([P, 8, 512], f32, tag="hT")
                    for mf in range(8):
                        ms = slice(mf * P, (mf + 1) * P)
                        ps1 = pp1.tile([P, 512], f32, tag="ps1")
                        for k in range(8):
                            nc.tensor.matmul(
                                ps1[:], rr(w1_sb[:, k, ms]), rr(xs[k][:, ts512]),
                                start=(k == 0), stop=(k == 7),
                            )
                        nc.scalar.activation(
                            rr(hT[:, mf, :]), ps1[:], Relu,
                            bias=b1c_sb[:, mf : mf + 1],
                        )
                    for t2 in range(4):
                        tg = nt * 4 + t2
                        for nf in range(2):
                            fs = slice(nf * 512, (nf + 1) * 512)
                            ps2 = pp2.tile([P, 512], f32, tag="ps2")
                            for k in range(8):
                                nc.tensor.matmul(
                                    ps2[:], rr(hT[:, k, t2 * P : (t2 + 1) * P]),
                                    rr(w2_sb[:, k, fs]), start=(k == 0), stop=False,
                                )
                            nc.tensor.matmul(
                                ps2[:], rr(ones_r[0:1, :]), rr(b2_r[0:1, fs]),
                                start=False, stop=True,
                            )
                            ot = op_.tile([P, 512], f32, tag="ot")
                            nc.vector.tensor_scalar_mul(
                                ot[:], ps2[:], m01c_sb[:, tg : tg + 1]
                            )
                            nc.sync.dma_start(
                                out_d[tg * P : (tg + 1) * P, fs], ot[:]
                            )

    nc.compile()
    return nc


def _get_nc():
    if "nc" not in _CACHE:
        _CACHE["nc"] = _build_nc()
    return _CACHE["nc"]


def _prep_inputs(inputs):
    ctx_all = np.ascontiguousarray(np.asarray(inputs["context_info"], np.float32))
    q_all = np.ascontiguousarray(np.asarray(inputs["query_info"], np.float32))
    cmask = np.asarray(inputs["context_mask"])
    qmask = np.asarray(inputs["query_mask"])
    w_sim = np.asarray(inputs["w_sim"], np.float32)
    w1 = np.ascontiguousarray(np.asarray(inputs["w_inner"], np.float32))
    b1c = np.ascontiguousarray(
        np.asarray(inputs["b_inner"], np.float32).reshape(8, P).T
    )
    w2 = np.ascontiguousarray(np.asarray(inputs["w_out"], np.float32))
    b2 = np.asarray(inputs["b_out"], np.float32).reshape(1, D8).copy()
    # b_sim is dropped: an additive constant cancels in both softmaxes.
    wc = np.ascontiguousarray(w_sim[:D].reshape(2, P).T)
    wq = np.ascontiguousarray(w_sim[D : 2 * D].reshape(2, P).T)
    wx = np.ascontiguousarray(w_sim[2 * D :].reshape(2, P).T)
    in_maps = []
    for b in range(B):
        cma = np.where(cmask[b], NEG, np.float32(0)).astype(np.float32)
        m01 = np.where(cmask[b], np.float32(0), np.float32(1)).astype(np.float32)
        in_maps.append(
            {
                "ctxT": np.ascontiguousarray(ctx_all[b].T),
                "ctx": ctx_all[b],
                "q": q_all[b],
                "qT": np.ascontiguousarray(q_all[b].T),
                "cma2": np.ascontiguousarray(cma.reshape(NCH, P).T),
                "m01c": np.ascontiguousarray(m01.reshape(NCH, P).T),
                "qma": np.where(qmask[b], NEG, np.float32(0))
                .astype(np.float32)
                .reshape(1, LQ),
                "wc": wc,
                "wq": wq,
                "wx": wx,
                "w1": w1,
                "b1c": b1c,
                "w2": w2,
                "b2": b2,
            }
        )
    return in_maps


def run(inputs, **kwargs):
    from concourse.bass_utils import run_bass_kernel_spmd

    nc = _get_nc()
    in_maps = _prep_inputs(inputs)
    res = run_bass_kernel_spmd(nc, in_maps, core_ids=list(range(B)), **kwargs)
    out = np.stack([r["out"] for r in res.results], axis=0)
    return out, res


def kernel(**inputs):
    out, _ = run(inputs)
    return out


# revision 8
# speedup vs baseline: 1.1521x; 1.0272x over previous
"""Trainium2 Bass kernel for nn_AttentionFlow_Layer (BiDAF-style attention flow).

Sharding: data-parallel over batch B=8 — one batch element per NeuronCore, no
collectives. Per core the computation is

  sim[i,j] = ctx[i]·wc + q[j]·wq + (ctx[i]*wx)·q[j] (+ b_sim, which cancels in
             both softmaxes and is dropped) with -1e10 additive masks
  c2q      = softmax_j(sim) @ q
  q2c      = softmax_i(max_j sim) @ ctx
  x        = [ctx | c2q | ctx*c2q | ctx*q2c]            # [2048, 1024]
  out      = (relu(x @ W1 + b1) @ W2 + b2) * (1 - context_mask)

Activations are kept feature-major ("transposed", [d, tokens]) so both FFN
matmuls use the weight matrices directly as the stationary operand:
  hT = relu(W1.T @ xT + b1),  out[t, f] = hT[:, t]·W2[:, f] + b2
Row masking is applied once, at the final PSUM eviction, as a per-partition
(token) scalar multiply — masked rows come out exactly 0.

fp32 data throughout. Large matmuls run as float32r (single-pass full-rate
fp32): every f32r operand is produced by a DVE/ACT instruction whose output
access pattern is typed float32r, which applies the hardware rounding the BIR
verifier demands. Attention-statistics matmuls stay full fp32.
"""

import numpy as np

B, LC, LQ = 8, 2048, 128
D, D8 = 256, 1024
P = 128
NCH = LC // P  # 16 context chunks
NEG = np.float32(-1e10)
CLAMP = -30.0

_CACHE = {}


def _build_nc():
    import concourse.mybir as mybir
    import concourse.tile as tile
    from concourse import bacc
    from concourse.masks import make_identity

    f32 = mybir.dt.float32
    f32r = mybir.dt.float32r
    X = mybir.AxisListType.X
    Exp = mybir.ActivationFunctionType.Exp
    Relu = mybir.ActivationFunctionType.Relu
    Alu = mybir.AluOpType

    nc = bacc.Bacc("TRN2", target_bir_lowering=False, debug=False)

    # ---- DRAM I/O (per-core shard) ----
    ctxT_d = nc.dram_tensor("ctxT", [D, LC], f32, kind="ExternalInput")
    ctx_d = nc.dram_tensor("ctx", [LC, D], f32, kind="ExternalInput")
    q_d = nc.dram_tensor("q", [LQ, D], f32, kind="ExternalInput")
    qT_d = nc.dram_tensor("qT", [D, LQ], f32, kind="ExternalInput")
    cma2_d = nc.dram_tensor("cma2", [P, NCH], f32, kind="ExternalInput")
    m01c_d = nc.dram_tensor("m01c", [P, NCH], f32, kind="ExternalInput")
    qma_d = nc.dram_tensor("qma", [1, LQ], f32, kind="ExternalInput")
    wc2_d = nc.dram_tensor("wc2", [P, 2, 2], f32, kind="ExternalInput")
    wq_d = nc.dram_tensor("wq", [P, 2], f32, kind="ExternalInput")
    wx_d = nc.dram_tensor("wx", [P, 2], f32, kind="ExternalInput")
    w1_d = nc.dram_tensor("w1", [D8, D8], f32, kind="ExternalInput")
    b1c_d = nc.dram_tensor("b1c", [P, 8], f32, kind="ExternalInput")
    w2_d = nc.dram_tensor("w2", [D8, D8], f32, kind="ExternalInput")
    b2_d = nc.dram_tensor("b2", [1, D8], f32, kind="ExternalInput")
    out_d = nc.dram_tensor("out", [LC, D8], f32, kind="ExternalOutput")

    def rr(ap):
        # float32r view of an f32 tile. Used on producer *outputs* (applies
        # hw rounding) and matmul operand inputs.
        return ap.bitcast(f32r)

    with tile.TileContext(nc) as tc:
        with tc.tile_pool(name="persist", bufs=1) as pp:
            ones = pp.tile([P, P], f32, tag="ones")
            nc.vector.memset(ones[:], 1.0)
            ident = pp.tile([P, P], f32, tag="ident")
            make_identity(nc, ident[:])

            # f32r-rounded FFN/attention operands (filled by copy/compute below)
            ctxT_r = pp.tile([P, 2, LC], f32, tag="ctxT_r")
            qx = pp.tile([P, 2, LQ], f32, tag="qx")
            c2qT = pp.tile([P, 2, LC], f32, tag="c2qT")
            xb2 = pp.tile([P, 2, LC], f32, tag="xb2")
            xb3 = pp.tile([P, 2, LC], f32, tag="xb3")
            w1_sb = pp.tile([P, 8, D8], f32, tag="w1")
            w2_sb = pp.tile([P, 8, D8], f32, tag="w2")
            ones_r = pp.tile([1, P], f32, tag="ones_r")
            nc.vector.tensor_copy(rr(ones_r[:]), ones[0:1, :])
            b2_r = pp.tile([1, D8], f32, tag="b2_r")

            m01c_sb = pp.tile([P, NCH], f32, tag="m01c")
            nc.sync.dma_start(m01c_sb[:], m01c_d[:])
            b1c_sb = pp.tile([P, 8], f32, tag="b1c")
            nc.sync.dma_start(b1c_sb[:], b1c_d[:])

            # ---------------- loads + f32r rounding copies ----------------
            # Order matters: attention inputs first so the PE can start while
            # the 8 MB of FFN weights stream in behind them.
            with (
                tc.tile_pool(name="wstage", bufs=2) as ws,
                tc.tile_pool(name="att_c", bufs=1) as ac,
            ):
                # PE warmup in the DMA shadow: ~10us of throwaway matmuls so
                # the HAM clock gate reaches 8/8 before the real work lands.
                with tc.tile_pool(name="ps_w", bufs=1, space="PSUM") as psw:
                    ps_warm = psw.tile([P, P], f32, tag="ps_warm")
                    for _ in range(24):
                        nc.tensor.matmul(
                            ps_warm[:], ones[:], ident[:], start=True, stop=True
                        )

                qT_sb = ac.tile([P, 2, LQ], f32, tag="qT_sb")
                nc.sync.dma_start(
                    qT_sb[:], qT_d[:].rearrange("(o p) j -> p o j", p=P)
                )
                q_sb = ac.tile([P, 2, P], f32, tag="q_sb")
                nc.sync.dma_start(q_sb[:], q_d[:].rearrange("j (h d) -> j h d", d=P))
                qma_sb = ac.tile([1, LQ], f32, tag="qma_sb")
                nc.sync.dma_start(qma_sb[:], qma_d[:])
                cma2_sb = ac.tile([P, NCH], f32, tag="cma2_sb")
                nc.sync.dma_start(cma2_sb[:], cma2_d[:])
                wc2_st = ac.tile([P, 2, 2], f32, tag="wc2_st")
                nc.sync.dma_start(wc2_st[:], wc2_d[:])
                wc2_r = ac.tile([P, 2, 2], f32, tag="wc2_r")
                nc.vector.tensor_copy(rr(wc2_r[:]), wc2_st[:])
                wq_sb = ac.tile([P, 2], f32, tag="wq_sb")
                nc.sync.dma_start(wq_sb[:], wq_d[:])
                wx_sb = ac.tile([P, 2], f32, tag="wx_sb")
                nc.sync.dma_start(wx_sb[:], wx_d[:])
                q_r = ac.tile([P, 2, P], f32, tag="q_r")
                for h in range(2):
                    nc.vector.tensor_copy(rr(q_r[:, h, :]), q_sb[:, h, :])
                ctxT_st = ac.tile([P, 2, LC], f32, tag="ctxT_st")
                nc.sync.dma_start(
                    ctxT_st[:], ctxT_d[:].rearrange("(o p) i -> p o i", p=P)
                )
                for kd in range(2):
                    nc.vector.tensor_copy(rr(ctxT_r[:, kd, :]), ctxT_st[:, kd, :])
                # qx = qT * wx, rounded
                for kd in range(2):
                    nc.vector.tensor_scalar_mul(
                        rr(qx[:, kd, :]), qT_sb[:, kd, :], wx_sb[:, kd : kd + 1]
                    )

                ctx_sb = ac.tile([P, NCH, D], f32, tag="ctx_sb")
                nc.sync.dma_start(
                    ctx_sb[:], ctx_d[:].rearrange("(o p) d -> p o d", p=P)
                )

                # ---------------- attention ----------------
                with (
                    tc.tile_pool(name="att_w", bufs=2) as aw,
                    tc.tile_pool(name="ps_s", bufs=2, space="PSUM") as pss,
                    tc.tile_pool(name="ps_t", bufs=2, space="PSUM") as pst,
                    tc.tile_pool(name="ps_cq", bufs=2, space="PSUM") as pscq,
                    tc.tile_pool(name="ps_sm", bufs=2, space="PSUM") as psm,
                ):
                    # sq' = q @ wq + qmask_add  -> [1, LQ]   (full fp32)
                    ps_sq = psm.tile([P, 512], f32, tag="psmall", name="ps_sq")
                    nc.tensor.matmul(
                        ps_sq[0:1, :LQ], wq_sb[:, 0:1], qT_sb[:, 0, :],
                        start=True, stop=False,
                    )
                    nc.tensor.matmul(
                        ps_sq[0:1, :LQ], wq_sb[:, 1:2], qT_sb[:, 1, :],
                        start=False, stop=False,
                    )
                    nc.tensor.matmul(
                        ps_sq[0:1, :LQ], ones[0:1, 0:1], qma_sb[0:1, :],
                        start=False, stop=True,
                    )
                    sqp = aw.tile([1, LQ], f32, tag="sqp", bufs=1)
                    nc.scalar.copy(rr(sqp[:]), ps_sq[0:1, :LQ])

                    scm = aw.tile([P, NCH], f32, tag="scm", bufs=1)
                    m_all = aw.tile([P, NCH], f32, tag="m_all", bufs=1)
                    s_all = aw.tile([P, NCH], f32, tag="s_all", bufs=1)
                    rec_all = aw.tile([P, NCH], f32, tag="rec_all", bufs=1)

                    atile = None
                    for c in range(NCH):
                        cs = slice(c * P, (c + 1) * P)
                        # sc + cmask_add, column layout [128, 1] (f32r;
                        # wc2 col 1 is zeros so the dst free size stays even)
                        ps_sc = psm.tile([P, 512], f32, tag="psmall", name="ps_sc")
                        nc.tensor.matmul(
                            ps_sc[:, 0:2], rr(ctxT_r[:, 0, cs]), rr(wc2_r[:, 0, :]),
                            start=True, stop=False,
                        )
                        nc.tensor.matmul(
                            ps_sc[:, 0:2], rr(ctxT_r[:, 1, cs]), rr(wc2_r[:, 1, :]),
                            start=False, stop=True,
                        )
                        nc.vector.tensor_tensor(
                            scm[:, c : c + 1], ps_sc[:, 0:1], cma2_sb[:, c : c + 1],
                            Alu.add,
                        )

                        # sim chunk [128 i, 128 j] = ctxT'.T @ qx + sq'[j]
                        ps_s = pss.tile([P, P], f32, tag="ps_s")
                        nc.tensor.matmul(
                            ps_s[:], rr(ctxT_r[:, 0, cs]), rr(qx[:, 0, :]),
                            start=True, stop=False,
                        )
                        nc.tensor.matmul(
                            ps_s[:], rr(ctxT_r[:, 1, cs]), rr(qx[:, 1, :]),
                            start=False, stop=False,
                        )
                        nc.tensor.matmul(
                            ps_s[:], rr(ones_r[0:1, :]), rr(sqp[0:1, :]),
                            start=False, stop=True,
                        )
                        # + sc[i] (per-partition), clamp at -30
                        smc = aw.tile([P, P], f32, tag="smc")
                        nc.vector.tensor_scalar(
                            smc[:], ps_s[:], scm[:, c : c + 1], CLAMP, Alu.add, Alu.max
                        )
                        nc.vector.reduce_max(m_all[:, c : c + 1], smc[:], axis=X)
                        U = aw.tile([P, P], f32, tag="U")
                        nc.scalar.activation(
                            U[:], smc[:], Exp, accum_out=s_all[:, c : c + 1]
                        )
                        nc.vector.reciprocal(rec_all[:, c : c + 1], s_all[:, c : c + 1])
                        A = aw.tile([P, P], f32, tag="A")
                        nc.vector.tensor_scalar_mul(A[:], U[:], rec_all[:, c : c + 1])
                        ps_t = pst.tile([P, P], f32, tag="ps_t")
                        nc.tensor.transpose(ps_t[:], A[:], ident[:])
                        g, cc = divmod(c, 4)
                        if cc == 0:
                            atile = aw.tile([P, 512], f32, tag="atile")
                        nc.scalar.copy(rr(atile[:, cc * P : (cc + 1) * P]), ps_t[:])
                        if cc == 3:
                            gs = slice(g * 512, (g + 1) * 512)
                            for h in range(2):
                                ps_cq = pscq.tile([P, 512], f32, tag="ps_cq")
                                nc.tensor.matmul(
                                    ps_cq[:], rr(q_r[:, h, :]), rr(atile[:]),
                                    start=True, stop=True,
                                )
                                nc.vector.tensor_copy(rr(c2qT[:, h, gs]), ps_cq[:])

                    # ---- q2c: softmax_i(max_j sim) @ ctx (full fp32) ----
                    E = aw.tile([P, NCH], f32, tag="E", bufs=1)
                    Es = aw.tile([P, 1], f32, tag="Es", bufs=1)
                    nc.scalar.activation(E[:], m_all[:], Exp, accum_out=Es[:])
                    ps_S = psm.tile([P, 512], f32, tag="psmall", name="ps_S")
                    nc.tensor.matmul(
                        ps_S[0:1, 0:1], Es[:], ones[:, 0:1], start=True, stop=True
                    )
                    invS = aw.tile([1, 1], f32, tag="invS", bufs=1)
                    nc.vector.reciprocal(invS[:], ps_S[0:1, 0:1])

                    ps_q2c = psm.tile([P, 512], f32, tag="psmall", name="ps_q2c")
                    for c in range(NCH):
                        nc.tensor.matmul(
                            ps_q2c[0:1, :D], E[:, c : c + 1], ctx_sb[:, c, :],
                            start=(c == 0), stop=(c == NCH - 1),
                        )
                    q2cn = aw.tile([1, D], f32, tag="q2cn", bufs=1)
                    nc.vector.tensor_scalar_mul(q2cn[:], ps_q2c[0:1, :D], invS[:])

                    ps_qt = pst.tile([P, P], f32, tag="ps_t", name="ps_qt")
                    for h in range(2):
                        nc.tensor.matmul(
                            ps_qt[:, h : h + 1], q2cn[0:1, h * P : (h + 1) * P],
                            ones[0:1, 0:1], start=True, stop=True,
                        )
                    q2cT = aw.tile([P, 2], f32, tag="q2cT", bufs=1)
                    nc.scalar.copy(q2cT[:], ps_qt[:, 0:2])

                    # ---- x blocks 2/3 (rounded for the FFN) ----
                    for kd in range(2):
                        nc.vector.tensor_tensor(
                            rr(xb2[:, kd, :]), ctxT_r[:, kd, :], c2qT[:, kd, :],
                            Alu.mult,
                        )
                        nc.vector.tensor_scalar_mul(
                            rr(xb3[:, kd, :]), ctxT_r[:, kd, :], q2cT[:, kd : kd + 1]
                        )

                # FFN weights: loaded after the attention inputs so their DMA
                # and rounding casts never get ahead of the attention pipeline.
                b2_st = ac.tile([1, D8], f32, tag="b2_st")
                nc.sync.dma_start(b2_st[:], b2_d[:])
                nc.vector.tensor_copy(rr(b2_r[:]), b2_st[:])
                for k in range(8):
                    w1_st = ws.tile([P, D8], f32, tag="w1_st")
                    nc.sync.dma_start(w1_st[:], w1_d[k * P : (k + 1) * P, :])
                    nc.vector.tensor_copy(rr(w1_sb[:, k, :]), w1_st[:])
                for k in range(8):
                    w2_st = ws.tile([P, D8], f32, tag="w2_st")
                    nc.sync.dma_start(w2_st[:], w2_d[k * P : (k + 1) * P, :])
                    nc.scalar.copy(rr(w2_sb[:, k, :]), w2_st[:])

            # ---------------- FFN ----------------
            xs = [
                ctxT_r[:, 0, :], ctxT_r[:, 1, :],
                c2qT[:, 0, :], c2qT[:, 1, :],
                xb2[:, 0, :], xb2[:, 1, :],
                xb3[:, 0, :], xb3[:, 1, :],
            ]
            with (
                tc.tile_pool(name="ffn_sb", bufs=2) as fp,
                tc.tile_pool(name="ffn_ot", bufs=3) as op_,
                tc.tile_pool(name="psum1", bufs=3, space="PSUM") as pp1,
                tc.tile_pool(name="psum2", bufs=3, space="PSUM") as pp2,
            ):
                for nt in range(4):
                    ts512 = slice(nt * 512, (nt + 1) * 512)
                    hT = fp.tile([P, 8, 512], f32, tag="hT")
                    for mf in range(8):
                        ms = slice(mf * P, (mf + 1) * P)
                        ps1 = pp1.tile([P, 512], f32, tag="ps1")
                        for k in range(8):
                            nc.tensor.matmul(
                                ps1[:], rr(w1_sb[:, k, ms]), rr(xs[k][:, ts512]),
                                start=(k == 0), stop=(k == 7),
                            )
                        nc.scalar.activation(
                            rr(hT[:, mf, :]), ps1[:], Relu,
                            bias=b1c_sb[:, mf : mf + 1],
                        )
                    for t2 in range(4):
                        tg = nt * 4 + t2
                        for nf in range(2):
                            fs = slice(nf * 512, (nf + 1) * 512)
                            ps2 = pp2.tile([P, 512], f32, tag="ps2")
                            for k in range(8):
                                nc.tensor.matmul(
                                    ps2[:], rr(hT[:, k, t2 * P : (t2 + 1) * P]),
                                    rr(w2_sb[:, k, fs]), start=(k == 0), stop=False,
                                )
                            nc.tensor.matmul(
                                ps2[:], rr(ones_r[0:1, :]), rr(b2_r[0:1, fs]),
                                start=False, stop=True,
                            )
                            ot = op_.tile([P, 512], f32, tag="ot")
                            nc.vector.tensor_scalar_mul(
                                ot[:], ps2[:], m01c_sb[:, tg : tg + 1]
                            )
                            nc.sync.dma_start(
                                out_d[tg * P : (tg + 1) * P, fs], ot[:]
                            )

    nc.compile()
    return nc


def _get_nc():
    if "nc" not in _CACHE:
        _CACHE["nc"] = _build_nc()
    return _CACHE["nc"]


def _prep_inputs(inputs):
    ctx_all = np.ascontiguousarray(np.asarray(inputs["context_info"], np.float32))
    q_all = np.ascontiguousarray(np.asarray(inputs["query_info"], np.float32))
    cmask = np.asarray(inputs["context_mask"])
    qmask = np.asarray(inputs["query_mask"])
    w_sim = np.asarray(inputs["w_sim"], np.float32)
    w1 = np.ascontiguousarray(np.asarray(inputs["w_inner"], np.float32))
    b1c = np.ascontiguousarray(
        np.asarray(inputs["b_inner"], np.float32).reshape(8, P).T
    )
    w2 = np.ascontiguousarray(np.asarray(inputs["w_out"], np.float32))
    b2 = np.asarray(inputs["b_out"], np.float32).reshape(1, D8).copy()
    # b_sim is dropped: an additive constant cancels in both softmaxes.
    wc = w_sim[:D].reshape(2, P).T  # [128, 2]
    wc2 = np.zeros((P, 2, 2), np.float32)
    wc2[:, :, 0] = wc
    wc2 = np.ascontiguousarray(wc2)
    wq = np.ascontiguousarray(w_sim[D : 2 * D].reshape(2, P).T)
    wx = np.ascontiguousarray(w_sim[2 * D :].reshape(2, P).T)
    in_maps = []
    for b in range(B):
        cma = np.where(cmask[b], NEG, np.float32(0)).astype(np.float32)
        m01 = np.where(cmask[b], np.float32(0), np.float32(1)).astype(np.float32)
        in_maps.append(
            {
                "ctxT": np.ascontiguousarray(ctx_all[b].T),
                "ctx": ctx_all[b],
                "q": q_all[b],
                "qT": np.ascontiguousarray(q_all[b].T),
                "cma2": np.ascontiguousarray(cma.reshape(NCH, P).T),
                "m01c": np.ascontiguousarray(m01.reshape(NCH, P).T),
                "qma": np.where(qmask[b], NEG, np.float32(0))
                .astype(np.float32)
                .reshape(1, LQ),
                "wc2": wc2,
                "wq": wq,
                "wx": wx,
                "w1": w1,
                "b1c": b1c,
                "w2": w2,
                "b2": b2,
            }
        )
    return in_maps


def run(inputs, **kwargs):
    from concourse.bass_utils import run_bass_kernel_spmd

    nc = _get_nc()
    in_maps = _prep_inputs(inputs)
    res = run_bass_kernel_spmd(nc, in_maps, core_ids=list(range(B)), **kwargs)
    out = np.stack([r["out"] for r in res.results], axis=0)
    return out, res


def kernel(**inputs):
    out, _ = run(inputs)
    return out
